# revision 5
# baseline (speedup 1.0000x reference)
"""Biaffine labeler kernel for 8x Trainium2 NeuronCores (v3).

Full-input contract: kernel(**inputs) takes the unsharded inputs and
returns the full [8, 256, 50] float32 logits.

Sharding: data-parallel over B - core i handles batch i. The projection
weights, bilinear tensor W, and biases are replicated.

Per-core pipeline (T=256 tokens, D=1024, DL=512, NL=50 labels):
  1. head_label = head[b] @ Whead          (PE, bf16, K=1024)
     headpack streamed in 8 single-k chunks split across two DMA
     queues (sync + scalar) so the PE starts ~8.5us in.
  2. one-hot(idx) built on DVE; sel^T = head_label^T gathered via a
     one-hot matmul on PE; bhead added on ACT during PSUM->SBUF copy.
     Two copies per chunk: bf16 (e3m4 labels) and e4m3 x16 (DR labels).
  3. label group matmuls:
     - 42 labels: selT bf16 stationary x W e3m4 moving (scale 202,
       max|202*W| = 15.46 < 15.5 by the xavier bound), 1 cyc/row.
     - 8 labels: DoubleRow fp8: selT e4m3 x16 stationary x W e4m3
       x3100 moving, 2x PE rate (measured 109ns per K=256/256-col
       instr vs 221ns for bf16 512-col).
  4. dep_label = dep[b] @ Wdep + bdep      (PE bf16 + DVE add),
     deppack streamed on the vector queue.
  5. logits[t,n] = sum_d dep_label[t,d] * P_n[t,d] / scale via DVE
     scalar_tensor_tensor accum_out; label bias added at the end.
  6. single contiguous output DMA: fin [128, 2*64] -> out [128, 128].

W streams from HBM on the gpsimd-engine DMA queue in label-group
chunks prefetched 4 deep; input packs ride sync/scalar/vector queues.
"""

import sys

sys.path.insert(0, "/opt/trn_rl_repo")

import numpy as np
import ml_dtypes

B, T, D = 8, 256, 1024
NL, DL = 50, 512

N_DR = 8                      # labels on the fp8 DoubleRow path
N_E3 = NL - N_DR              # labels on the e3m4 path
SW3 = 202.0                   # e3m4 W scale (202*0.07655 = 15.46 < 15.5)
SS4 = 16.0                    # e4m3 sel scale
SW4 = 3100.0                  # e4m3 W scale (3100*0.07655 = 237.3 < 240)

GROUPS3 = [1, 1, 2, 2] + [4] * 8 + [2, 1, 1]   # 42
GROUPS4 = [4, 4]                               # 8
assert sum(GROUPS3) == N_E3 and sum(GROUPS4) == N_DR

GF = 4 * 4 * DL               # free-dim elems per 4-label W chunk (8192)
PREFETCH = 4

# constpack layout (f32 columns)
C_IDX = 0            # [128, 256] idx broadcast
C_BDEP = 256         # [128, 512] bdep broadcast
C_BIAS = 768         # [128, 50] label bias broadcast
C_IOTA = 818         # [128, 2] iota columns
C_BHEAD = 820        # [128, 4] bhead chunks
C_BHEAD16 = 824      # [128, 4] 16*bhead chunks
C_TOT = 828

BF16 = ml_dtypes.bfloat16
E3 = ml_dtypes.float8_e3m4
E4 = ml_dtypes.float8_e4m3

HPC = 768   # headpack chunk cols: [whead_k (512) | headT_k (256)]
DPC = 1536  # deppack chunk cols: [w 2c | w 2c+1 | xT 2c | xT 2c+1]

LAST_RESULTS = None
_NC_CACHE = None


def _schedule():
    """Build the group schedule: (kind, slot0, nlabels, dram_index)."""
    sched = []
    n0 = 0
    for gi, sz in enumerate(GROUPS3):
        sched.append(("e3", n0, sz, gi))
        n0 += sz
    n4 = N_E3
    for gi, sz in enumerate(GROUPS4):
        sched.append(("e4", n4, sz, gi))
        n4 += sz
    # order: first 4 e3 groups, both e4 groups, remaining e3 groups
    order = sched[:4] + sched[len(GROUPS3):] + sched[4:len(GROUPS3)]
    return order


def _build_nc():
    import concourse.bacc as bacc
    import concourse.mybir as mybir
    import concourse.tile as tile

    bf = mybir.dt.bfloat16
    f32 = mybir.dt.float32
    e3 = mybir.dt.float8e3
    e4 = mybir.dt.float8e4
    Alu = mybir.AluOpType
    Act = mybir.ActivationFunctionType
    DR = mybir.MatmulPerfMode.DoubleRow

    nc = bacc.Bacc(None)

    hp = nc.dram_tensor("hp", [128, 8 * HPC], bf, kind="ExternalInput")
    dp = nc.dram_tensor("dp", [128, 4 * DPC], bf, kind="ExternalInput")
    cp = nc.dram_tensor("cp", [128, C_TOT], f32, kind="ExternalInput")
    wg3 = nc.dram_tensor("wg3", [len(GROUPS3), 128, GF], e3, kind="ExternalInput")
    wg4 = nc.dram_tensor("wg4", [len(GROUPS4), 128, GF], e4, kind="ExternalInput")
    out = nc.dram_tensor("out", [128, 128], f32, kind="ExternalOutput")

    sched = _schedule()

    with tile.TileContext(nc) as tc:
        with (
            tc.sbuf_pool(name="cpool", bufs=1) as cpool,
            tc.sbuf_pool(name="persist", bufs=1) as pers,
            tc.sbuf_pool(name="wpool", bufs=PREFETCH + 1) as wpool,
            tc.sbuf_pool(name="spool", bufs=4) as spool,
            tc.psum_pool(name="ps", bufs=8) as ps,
        ):
            # --- input DMAs across four queues ---------------------------
            hp_c = [cpool.tile([128, HPC], bf, name=f"hp{k}") for k in range(8)]
            for k in range(4):
                nc.sync.dma_start(hp_c[k][:], hp[:, k * HPC : (k + 1) * HPC])

            cp_sb = cpool.tile([128, C_TOT], f32)
            nc.scalar.dma_start(cp_sb[:], cp[:])
            for k in range(4, 8):
                nc.scalar.dma_start(hp_c[k][:], hp[:, k * HPC : (k + 1) * HPC])

            # fin zero-filled early so the tail DMA can ship all 128 cols
            fin = pers.tile([128, 128], f32, tag="fin", name="fin")
            nc.vector.memset(fin[:], 0.0)

            wg_tiles = {}

            def issue_wg(si):
                kind, n0, ng, gi = sched[si]
                if kind == "e3":
                    wt = wpool.tile([128, GF], e3, tag="wg", name=f"w3_{gi}")
                    nc.gpsimd.dma_start(
                        wt[:, : ng * 4 * 512], wg3[gi, :, : ng * 4 * 512]
                    )
                else:
                    wt = wpool.tile([128, GF], e4, tag="wg", name=f"w4_{gi}")
                    nc.gpsimd.dma_start(
                        wt[:, : ng * 4 * 512], wg4[gi, :, : ng * 4 * 512]
                    )
                wg_tiles[si] = wt

            issue_wg(0)
            dp_c = [cpool.tile([128, DPC], bf, name=f"dp{c}") for c in range(4)]
            for c in range(4):
                nc.gpsimd.dma_start(dp_c[c][:], dp[:, c * DPC : (c + 1) * DPC])
            for si in range(1, PREFETCH + 1):
                issue_wg(si)

            # --- head projection: head_label[j] = [128 t, 512 d] ---------
            ph = [ps.tile([128, 512], f32, tag="ps", name=f"ph{j}") for j in range(2)]
            for k in range(8):
                for j in range(2):
                    nc.tensor.matmul(
                        ph[j][:],
                        lhsT=hp_c[k][:, 512 + j * 128 : 512 + j * 128 + 128],
                        rhs=hp_c[k][:, 0:512],
                        start=(k == 0),
                        stop=(k == 7),
                    )
            head_label = []
            for j in range(2):
                hlj = pers.tile([128, 512], bf, tag=f"hl{j}", name=f"hl{j}")
                nc.scalar.copy(hlj[:], ph[j][:])
                head_label.append(hlj)

            # --- one-hot of head_indices ---------------------------------
            onehot = []
            for j in range(2):
                ohj = pers.tile([128, 256], bf, tag=f"oh{j}", name=f"oh{j}")
                nc.vector.tensor_scalar(
                    out=ohj[:],
                    in0=cp_sb[:, C_IDX : C_IDX + 256],
                    scalar1=cp_sb[:, C_IOTA + j : C_IOTA + j + 1],
                    scalar2=None,
                    op0=Alu.is_equal,
                )
                onehot.append(ohj)

            # --- gather: selT[c] bf16 + s4[:, c, :] e4m3 -----------------
            selT = []
            s4 = pers.tile([128, 4, 256], e4, tag="s4", name="s4")
            for c in range(4):
                pg = ps.tile([128, 256], f32, tag="ps", name=f"pg{c}")
                for j in range(2):
                    nc.tensor.matmul(
                        pg[:],
                        lhsT=head_label[j][:, c * 128 : (c + 1) * 128],
                        rhs=onehot[j][:],
                        start=(j == 0),
                        stop=(j == 1),
                    )
                sc = pers.tile([128, 256], bf, tag=f"sel{c}", name=f"sel{c}")
                nc.scalar.activation(
                    sc[:],
                    pg[:],
                    Act.Identity,
                    bias=cp_sb[:, C_BHEAD + c : C_BHEAD + c + 1],
                    scale=1.0,
                )
                selT.append(sc)
                nc.scalar.activation(
                    s4[:, c, :],
                    pg[:],
                    Act.Identity,
                    bias=cp_sb[:, C_BHEAD16 + c : C_BHEAD16 + c + 1],
                    scale=SS4,
                )

            # --- output accumulators -------------------------------------
            out_sb = []
            for m in range(2):
                om = pers.tile([128, 64], f32, tag=f"out{m}", name=f"out{m}")
                out_sb.append(om)

            dep_label = []
            deferred = []

            def drain_bank(n, m, pbt, inv_scale):
                prod = spool.tile([128, 512], f32, tag="prod", name=f"prod_{n}_{m}")
                nc.vector.scalar_tensor_tensor(
                    out=prod[:],
                    in0=pbt[:],
                    scalar=inv_scale,
                    in1=dep_label[m][:],
                    op0=Alu.mult,
                    op1=Alu.mult,
                    accum_out=out_sb[m][:, n : n + 1],
                )

            def do_group(si, defer_drain=False):
                kind, n0, ng, gi = sched[si]
                wg_sb = wg_tiles[si]
                for li in range(ng):
                    n = n0 + li
                    for m in range(2):
                        pbt = ps.tile(
                            [128, 512], f32, tag="ps", name=f"pb_{n}_{m}"
                        )
                        if kind == "e3":
                            for c in range(4):
                                nc.tensor.matmul(
                                    pbt[:],
                                    lhsT=selT[c][:, m * 128 : (m + 1) * 128],
                                    rhs=wg_sb[
                                        :, (li * 4 + c) * 512 : (li * 4 + c + 1) * 512
                                    ],
                                    start=(c == 0),
                                    stop=(c == 3),
                                )
                            inv = 1.0 / SW3
                        else:
                            # NOTE: start=True zeroes the WHOLE psum bank on
                            # HW (not just the out AP range), so only the
                            # first instruction of the bank may set it.
                            for kp in range(2):
                                for dh in range(2):
                                    base = li * 2048 + kp * 1024 + dh * 512
                                    rhs = wg_sb[:, base : base + 512].rearrange(
                                        "p (two f) -> p two f", two=2
                                    )
                                    nc.tensor.matmul(
                                        pbt[:, dh * 256 : dh * 256 + 256],
                                        lhsT=s4[:, 2 * kp : 2 * kp + 2,
                                                m * 128 : m * 128 + 128],
                                        rhs=rhs,
                                        start=(kp == 0 and dh == 0),
                                        stop=(kp == 1),
                                        perf_mode=DR,
                                        skip_group_check=True,
                                    )
                            inv = 1.0 / (SS4 * SW4)
                        if defer_drain:
                            deferred.append((n, m, pbt, inv))
                        else:
                            drain_bank(n, m, pbt, inv)
                if si + PREFETCH + 1 < len(sched):
                    issue_wg(si + PREFETCH + 1)

            do_group(0, defer_drain=True)

            # --- dep projection ------------------------------------------
            def dw_slab(c, k):
                return dp_c[c][:, (k % 2) * 512 : (k % 2) * 512 + 512]

            def dxT_slab(c, k, j):
                o = 1024 + (k % 2) * 256 + j * 128
                return dp_c[c][:, o : o + 128]

            pd = [ps.tile([128, 512], f32, tag="ps", name=f"pd{m}") for m in range(2)]
            for k in range(8):
                for m in range(2):
                    nc.tensor.matmul(
                        pd[m][:],
                        lhsT=dxT_slab(k // 2, k, m),
                        rhs=dw_slab(k // 2, k),
                        start=(k == 0),
                        stop=(k == 7),
                    )
            for m in range(2):
                dl = pers.tile([128, 512], f32, tag=f"dl{m}", name=f"dl{m}")
                nc.vector.tensor_tensor(
                    dl[:], pd[m][:], cp_sb[:, C_BDEP : C_BDEP + 512], Alu.add
                )
                dep_label.append(dl)

            for (n, m, pbt, inv) in deferred:
                drain_bank(n, m, pbt, inv)
            deferred.clear()

            for si in range(1, len(sched)):
                do_group(si)

            # --- bias add + single contiguous store ----------------------
            for m in range(2):
                nc.vector.tensor_tensor(
                    fin[:, m * 64 : m * 64 + NL],
                    out_sb[m][:, :NL],
                    cp_sb[:, C_BIAS : C_BIAS + NL],
                    Alu.add,
                )
            nc.sync.dma_start(out[:], fin[:])

    nc.finalize()
    return nc


def _stage_shared(Wdep, bdep, Whead, bhead, W, bias):
    """Host-side staging of the replicated tensors."""

    def pack_w(Wm):  # [1024, 512] -> [128, 8, 512] k-slab major
        return Wm.reshape(8, 128, 512).transpose(1, 0, 2)

    whead_h = pack_w(Whead)
    wdep_h = pack_w(Wdep)

    # e3m4 labels: WT[n, c, p, d] = W[n, d, c*128+p] scaled
    WT = np.ascontiguousarray(W.transpose(0, 2, 1)).reshape(NL, 4, 128, 512)
    WT3 = np.clip(WT * SW3, -15.5, 15.5)
    wg3_h = np.zeros((len(GROUPS3), 128, GF), dtype=E3)
    n0 = 0
    for g, sz in enumerate(GROUPS3):
        blk = WT3[n0 : n0 + sz]  # [sz, 4, 128, 512]
        wg3_h[g, :, : sz * 4 * 512] = (
            blk.transpose(2, 0, 1, 3).reshape(128, sz * 4 * 512).astype(E3)
        )
        n0 += sz

    # DoubleRow labels: block[p, i*256+j] = SW4*W[n, dh*256+j, (2kp+i)*128+p]
    WT4 = np.clip(WT * SW4, -240.0, 240.0)  # [n, c, p, d]
    wg4_h = np.zeros((len(GROUPS4), 128, GF), dtype=E4)
    for g, sz in enumerate(GROUPS4):
        for li in range(sz):
            n = N_E3 + n0_of_group4(g) + li
            blocks = []
            for kp in range(2):
                for dh in range(2):
                    blk = WT4[n, 2 * kp : 2 * kp + 2, :, dh * 256 : dh * 256 + 256]
                    # blk [i, p, j] -> [p, i*256+j]
                    blocks.append(blk.transpose(1, 0, 2).reshape(128, 512))
            wg4_h[g, :, li * 2048 : (li + 1) * 2048] = np.concatenate(
                blocks, axis=1
            ).astype(E4)

    constpack = np.zeros((128, C_TOT), dtype=np.float32)
    constpack[:, C_BDEP : C_BDEP + 512] = bdep[None, :]
    constpack[:, C_BIAS : C_BIAS + NL] = bias[None, :]
    constpack[:, C_IOTA] = np.arange(128, dtype=np.float32)
    constpack[:, C_IOTA + 1] = 128 + np.arange(128, dtype=np.float32)
    constpack[:, C_BHEAD : C_BHEAD + 4] = bhead.reshape(4, 128).T
    constpack[:, C_BHEAD16 : C_BHEAD16 + 4] = SS4 * bhead.reshape(4, 128).T

    return {
        "whead_h": whead_h,
        "wdep_h": wdep_h,
        "wg3": wg3_h,
        "wg4": wg4_h,
        "constpack_base": constpack,
    }


def n0_of_group4(g):
    return sum(GROUPS4[:g])


def _stage_core(shared, dep_b, head_b, idx_b):
    """Host-side staging of one batch's activations."""

    def pack_x(x):  # [256, 1024] -> [128, 8, 256] k-slab major
        return x.T.reshape(8, 128, 256).transpose(1, 0, 2)

    headT_h = pack_x(head_b)
    depT_h = pack_x(dep_b)

    # headpack: 8 chunks of [whead_k (512) | headT_k (256)]
    hp_chunks = []
    for k in range(8):
        hp_chunks.append(shared["whead_h"][:, k])
        hp_chunks.append(headT_h[:, k])
    hp_h = np.ascontiguousarray(np.concatenate(hp_chunks, axis=1)).astype(BF16)

    # deppack: 4 chunks of [w 2c | w 2c+1 | xT 2c | xT 2c+1]
    dp_chunks = []
    for c in range(4):
        dp_chunks.append(shared["wdep_h"][:, 2 * c])
        dp_chunks.append(shared["wdep_h"][:, 2 * c + 1])
        dp_chunks.append(depT_h[:, 2 * c])
        dp_chunks.append(depT_h[:, 2 * c + 1])
    dp_h = np.ascontiguousarray(np.concatenate(dp_chunks, axis=1)).astype(BF16)

    constpack = shared["constpack_base"].copy()
    constpack[:, C_IDX : C_IDX + 256] = idx_b.astype(np.float32)[None, :]
    return {"hp": hp_h, "dp": dp_h, "cp": constpack}


def kernel(dep, head, head_indices, mask, Wdep, bdep, Whead, bhead, W, bias):
    global LAST_RESULTS, _NC_CACHE
    from concourse.bass_utils import run_bass_kernel_spmd

    dep = np.asarray(dep, dtype=np.float32)
    head = np.asarray(head, dtype=np.float32)
    head_indices = np.asarray(head_indices)
    Wdep = np.asarray(Wdep, dtype=np.float32)
    bdep = np.asarray(bdep, dtype=np.float32)
    Whead = np.asarray(Whead, dtype=np.float32)
    bhead = np.asarray(bhead, dtype=np.float32)
    W = np.asarray(W, dtype=np.float32)
    bias = np.asarray(bias, dtype=np.float32)

    if _NC_CACHE is None:
        _NC_CACHE = _build_nc()
    nc = _NC_CACHE

    shared = _stage_shared(Wdep, bdep, Whead, bhead, W, bias)
    in_maps = []
    for b in range(B):
        m = {"wg3": shared["wg3"], "wg4": shared["wg4"]}
        m.update(_stage_core(shared, dep[b], head[b], head_indices[b]))
        in_maps.append(m)

    res = run_bass_kernel_spmd(nc, in_maps, list(range(B)))
    LAST_RESULTS = res
    outs = []
    for b in range(B):
        r = np.asarray(res.results[b]["out"], dtype=np.float32)  # [128, 128]
        outs.append(np.concatenate([r[:, 0:NL], r[:, 64 : 64 + NL]], axis=0))
    return np.stack(outs, axis=0)
